# revision 45
# baseline (speedup 1.0000x reference)
"""ChannelAttentionModule kernel for TRN2 (Bass/Tile), 8-core SPMD.

Computes sigmoid(mean_{hw}(x) @ W.T + b) for x:[16,128,256,256].

Sharding: data-parallel over batch, 2 images per core (64 MiB/core), no
collectives; host concatenates the per-core [2] outputs into [16,1,1,1].

Per-core dataflow (memory-bound; HBM read of the shard is the roofline):
- The shard is read as 2 MiB *address-contiguous* slabs [128, 4096]
  (partition p <- slab_off + p*4096). Contiguous reads sustain ~390+ GB/s
  vs ~340 GB/s for per-channel strided reads. Channels then span
  partition groups, so the host precomputes expanded per-slab weights
  wexp[p, s] = W[channel(p, s)]/HW (scale by 1/HW is exact, power of 2).
- Per-slab H*W-partial sums: DVE reduce_sum for even chunks, ACT
  activation(Copy, accum_out) for odd chunks, so neither engine caps the
  DMA rate (DVE alone at 1x f32 is marginal against an uncontended
  stream).
- Channel contraction runs *during* the stream: one tiny accumulating
  PE matmul per slab, ps[1,2] += wexp[:,s].T @ partials[:,:,s] in PSUM.
- Tail: sigmoid(ps + b) on ACT, 8-byte DMA out. The last slab is split
  into 4 sub-slabs so the final exposed reduce is small.
- All x DMAs are issued on the single SP HWDGE ring; big pool is
  8-deep (128 KiB/partition) to keep the HBM request queue full.

Measured (8 cores concurrent, HBM stack shared per core-pair at
~755 GB/s): best-case fleet ~178 µs/core, typical mean ~187 µs, worst
core ~215-220 µs when PJRT launch skew lets early cores win arbitration.
"""

import numpy as np

_B, _C, _HW = 16, 128, 65536  # batch, channels, H*W
_NCORES = 8
_BPC = _B // _NCORES  # batches per core = 2
_NCH = 16  # full-size chunks per batch (last one split finer, see _slabs)
_F = _HW // _NCH  # free-dim elements per full chunk
_SPLIT_LAST = True


def _slab_list(nch=_NCH, split_last=_SPLIT_LAST):
    """Per-batch slabs as (flat_offset, free_elems_per_partition).

    nch-1 full slabs, then the last slab split into 4 sub-slabs so the
    final exposed DVE reduce is ~1/4 the size.
    """
    total = _C * _HW
    full = total // nch
    ff = full // 128
    if split_last:
        slabs = [(s * full, ff) for s in range(nch - 1)]
        sub = full // 4
        for k in range(4):
            slabs.append(((nch - 1) * full + k * sub, ff // 4))
    else:
        slabs = [(s * full, ff) for s in range(nch)]
    return slabs


_SLABS = _slab_list()
_NSLAB = len(_SLABS)

_cached_nc = None


def _build_nc(bufs=8, dual_ring=False, act_offload=True, slabs=None, asserts=True):
    import concourse.bacc as bacc
    import concourse.tile as tile
    from concourse import mybir

    f32 = mybir.dt.float32
    nc = bacc.Bacc(
        "TRN2",
        target_bir_lowering=False,
        debug=False,
        num_devices=_NCORES,
        enable_asserts=asserts,
    )

    if slabs is None:
        slabs = _SLABS
    nslab = len(slabs)

    # x stored flat per batch; each slab s is a fully contiguous region
    # read as [128, f] with partition p <- slab_offset + p*f.
    x = nc.dram_tensor("x", [_BPC, _C * _HW], f32, kind="ExternalInput")
    # Per-slab expanded weights (mean scale folded in on host):
    # wexp[p, s] = W[channel of partition p in slab s] / HW
    wexp = nc.dram_tensor("wexp", [128, nslab], f32, kind="ExternalInput")
    bvec = nc.dram_tensor("bias", [1, 1], f32, kind="ExternalInput")
    out = nc.dram_tensor("out", [1, _BPC], f32, kind="ExternalOutput")

    with tile.TileContext(nc) as tc:
        with (
            tc.tile_pool(name="big", bufs=bufs) as big,
            tc.tile_pool(name="small", bufs=1) as small,
            tc.tile_pool(name="psum", bufs=1, space="PSUM") as psum,
        ):
            # Tiny loads go via SWDGE (gpsimd) so the HWDGE ring starts
            # streaming x chunks immediately.
            w_sb = small.tile([128, nslab], f32)
            nc.gpsimd.dma_start(out=w_sb[:], in_=wexp[:])
            b_sb = small.tile([1, 1], f32)
            nc.gpsimd.dma_start(out=b_sb[:], in_=bvec[:])

            partials = small.tile([128, _BPC, nslab], f32)
            ps = psum.tile([1, _BPC], f32)
            nchunk = 0
            for s, (off, f) in enumerate(slabs):
                for bi in range(_BPC):
                    t = big.tile([128, f], f32, tag="xtile")
                    # dual_ring alternates DMA issue between the SP and ACT
                    # HWDGE rings; measured worse than SP-only (A/B'd), so
                    # the default keeps everything on nc.sync.
                    eng = (
                        nc.sync
                        if (nchunk % 2 == 0 or not dual_ring)
                        else nc.scalar
                    )
                    eng.dma_start(
                        out=t[:],
                        in_=x[bi, off : off + 128 * f].rearrange(
                            "(p f) -> p f", f=f
                        ),
                    )
                    if act_offload and nchunk % 2 == 1:
                        nc.scalar.activation(
                            out=t[:],
                            in_=t[:],
                            func=mybir.ActivationFunctionType.Copy,
                            accum_out=partials[:, bi, s : s + 1],
                        )
                    else:
                        nc.vector.reduce_sum(
                            out=partials[:, bi, s : s + 1],
                            in_=t[:],
                            axis=mybir.AxisListType.X,
                        )
                    nchunk += 1
                # Accumulate this slab's weighted partition-contraction
                # into PSUM while the stream continues:
                # ps[0, b] += sum_p wexp[p, s] * partials[p, b, s]
                nc.tensor.matmul(
                    ps[:],
                    w_sb[:, s : s + 1],
                    partials[:, :, s],
                    start=(s == 0),
                    stop=(s == nslab - 1),
                )

            # sigmoid(att + bias); mean scale already folded into wexp
            res = small.tile([1, _BPC], f32)
            nc.scalar.activation(
                out=res[:],
                in_=ps[:],
                func=mybir.ActivationFunctionType.Sigmoid,
                bias=b_sb[:],
                scale=1.0,
            )
            nc.sync.dma_start(out=out[:], in_=res[:])

    nc.compile()
    return nc


def _prepare_in_maps(x, W, b, slabs=None):
    if slabs is None:
        slabs = _SLABS
    xs = np.ascontiguousarray(x, dtype=np.float32).reshape(_B, _C * _HW)
    b_col = np.ascontiguousarray(b, dtype=np.float32).reshape(1, 1)
    # wexp[p, s] = W[channel of partition p in slab s] / HW, where the
    # channel of partition p in slab (off, f) is (off + p*f) // HW.
    w_flat = np.asarray(W, dtype=np.float32).reshape(_C)
    p = np.arange(128)[:, None]
    off = np.array([o for o, _ in slabs])[None, :]
    f = np.array([fe for _, fe in slabs])[None, :]
    ch = (off + p * f) // _HW
    wexp = np.ascontiguousarray(w_flat[ch] / np.float32(_HW), dtype=np.float32)
    return [
        {
            "x": np.ascontiguousarray(xs[i * _BPC : (i + 1) * _BPC]),
            "wexp": wexp,
            "bias": b_col,
        }
        for i in range(_NCORES)
    ]


def _gather(results):
    outs = [np.asarray(results[i]["out"]).reshape(_BPC) for i in range(_NCORES)]
    return np.concatenate(outs, axis=0).reshape(_B, 1, 1, 1).astype(np.float32)


def kernel(x, W, b):
    from concourse.bass_utils import run_bass_kernel_spmd

    global _cached_nc
    if _cached_nc is None:
        _cached_nc = _build_nc()
    in_maps = _prepare_in_maps(x, W, b)
    res = run_bass_kernel_spmd(_cached_nc, in_maps, list(range(_NCORES)))
    return _gather(res.results)


# revision 47
# speedup vs baseline: 1.0077x; 1.0077x over previous
"""ChannelAttentionModule kernel for TRN2 (Bass/Tile), 8-core SPMD.

Computes sigmoid(mean_{hw}(x) @ W.T + b) for x:[16,128,256,256].

Sharding: data-parallel over batch, 2 images per core (64 MiB/core), no
collectives; host concatenates the per-core [2] outputs into [16,1,1,1].

Per-core dataflow (memory-bound; HBM read of the shard is the roofline):
- The shard is read as 2 MiB *address-contiguous* slabs [128, 4096]
  (partition p <- slab_off + p*4096). Contiguous reads sustain ~390+ GB/s
  vs ~340 GB/s for per-channel strided reads. Channels then span
  partition groups, so the host precomputes expanded per-slab weights
  wexp[p, s] = W[channel(p, s)]/HW (scale by 1/HW is exact, power of 2).
- Per-slab H*W-partial sums: DVE reduce_sum for even chunks, ACT
  activation(Copy, accum_out) for odd chunks, so neither engine caps the
  DMA rate (DVE alone at 1x f32 is marginal against an uncontended
  stream).
- Channel contraction runs *during* the stream: one tiny accumulating
  PE matmul per slab, ps[1,2] += wexp[:,s].T @ partials[:,:,s] in PSUM.
- Tail: sigmoid(ps + b) on ACT, 8-byte DMA out. The last slab is split
  into 4 sub-slabs so the final exposed reduce is small.
- All x DMAs are issued on the single SP HWDGE ring; big pool is
  8-deep (128 KiB/partition) to keep the HBM request queue full.

Measured (8 cores concurrent, HBM stack shared per core-pair at
~755 GB/s): best-case fleet ~178 µs/core, typical mean ~187 µs, worst
core ~215-220 µs when PJRT launch skew lets early cores win arbitration.
"""

import numpy as np

_B, _C, _HW = 16, 128, 65536  # batch, channels, H*W
_NCORES = 8
_BPC = _B // _NCORES  # batches per core = 2
_NCH = 16  # full-size chunks per batch (last one split finer, see _slabs)
_F = _HW // _NCH  # free-dim elements per full chunk
_SPLIT_LAST = True


def _slab_list(nch=_NCH, split_last=_SPLIT_LAST):
    """Per-batch slabs as (flat_offset, free_elems_per_partition).

    nch-1 full slabs, then the last slab split into 4 sub-slabs so the
    final exposed DVE reduce is ~1/4 the size.
    """
    total = _C * _HW
    full = total // nch
    ff = full // 128
    if split_last:
        slabs = [(s * full, ff) for s in range(nch - 1)]
        sub = full // 4
        for k in range(4):
            slabs.append(((nch - 1) * full + k * sub, ff // 4))
    else:
        slabs = [(s * full, ff) for s in range(nch)]
    return slabs


_SLABS = _slab_list()
_NSLAB = len(_SLABS)

_cached_nc = None


def _build_nc(bufs=8, dual_ring=False, act_offload=True, slabs=None, asserts=True):
    import concourse.bacc as bacc
    import concourse.tile as tile
    from concourse import mybir

    f32 = mybir.dt.float32
    nc = bacc.Bacc(
        "TRN2",
        target_bir_lowering=False,
        debug=False,
        num_devices=_NCORES,
        enable_asserts=asserts,
    )

    if slabs is None:
        slabs = _SLABS
    nslab = len(slabs)

    # x stored flat per batch; each slab s is a fully contiguous region
    # read as [128, f] with partition p <- slab_offset + p*f.
    x = nc.dram_tensor("x", [_BPC, _C * _HW], f32, kind="ExternalInput")
    # Per-slab expanded weights (mean scale folded in on host):
    # wexp[p, s] = W[channel of partition p in slab s] / HW
    wexp = nc.dram_tensor("wexp", [128, nslab], f32, kind="ExternalInput")
    bvec = nc.dram_tensor("bias", [1, 1], f32, kind="ExternalInput")
    out = nc.dram_tensor("out", [1, _BPC], f32, kind="ExternalOutput")

    with tile.TileContext(nc) as tc:
        with (
            tc.tile_pool(name="big", bufs=bufs) as big,
            tc.tile_pool(name="sub", bufs=8) as sub,
            tc.tile_pool(name="small", bufs=1) as small,
            tc.tile_pool(name="psum", bufs=1, space="PSUM") as psum,
        ):
            # Tiny loads go via SWDGE (gpsimd) so the HWDGE ring starts
            # streaming x chunks immediately.
            w_sb = small.tile([128, nslab], f32)
            nc.gpsimd.dma_start(out=w_sb[:], in_=wexp[:])
            b_sb = small.tile([1, 1], f32)
            nc.gpsimd.dma_start(out=b_sb[:], in_=bvec[:])

            partials = small.tile([128, _BPC, nslab], f32)
            ps = psum.tile([1, _BPC], f32)
            nchunk = 0
            for s, (off, f) in enumerate(slabs):
                for bi in range(_BPC):
                    # Sub-slabs get dedicated slots so their DMAs queue
                    # immediately at stream end instead of serializing
                    # behind the last full-chunk reduces' slot releases.
                    if f == _F:
                        t = big.tile([128, f], f32, tag="xtile")
                    else:
                        t = sub.tile([128, f], f32, tag="subtile")
                    # dual_ring alternates DMA issue between the SP and ACT
                    # HWDGE rings; measured worse than SP-only (A/B'd), so
                    # the default keeps everything on nc.sync.
                    eng = (
                        nc.sync
                        if (nchunk % 2 == 0 or not dual_ring)
                        else nc.scalar
                    )
                    eng.dma_start(
                        out=t[:],
                        in_=x[bi, off : off + 128 * f].rearrange(
                            "(p f) -> p f", f=f
                        ),
                    )
                    if act_offload and nchunk % 2 == 1:
                        nc.scalar.activation(
                            out=t[:],
                            in_=t[:],
                            func=mybir.ActivationFunctionType.Copy,
                            accum_out=partials[:, bi, s : s + 1],
                        )
                    else:
                        nc.vector.reduce_sum(
                            out=partials[:, bi, s : s + 1],
                            in_=t[:],
                            axis=mybir.AxisListType.X,
                        )
                    nchunk += 1
                # Accumulate this slab's weighted partition-contraction
                # into PSUM while the stream continues:
                # ps[0, b] += sum_p wexp[p, s] * partials[p, b, s]
                nc.tensor.matmul(
                    ps[:],
                    w_sb[:, s : s + 1],
                    partials[:, :, s],
                    start=(s == 0),
                    stop=(s == nslab - 1),
                )

            # sigmoid(att + bias); mean scale already folded into wexp
            res = small.tile([1, _BPC], f32)
            nc.scalar.activation(
                out=res[:],
                in_=ps[:],
                func=mybir.ActivationFunctionType.Sigmoid,
                bias=b_sb[:],
                scale=1.0,
            )
            nc.sync.dma_start(out=out[:], in_=res[:])

    nc.compile()
    return nc


def _prepare_in_maps(x, W, b, slabs=None):
    if slabs is None:
        slabs = _SLABS
    xs = np.ascontiguousarray(x, dtype=np.float32).reshape(_B, _C * _HW)
    b_col = np.ascontiguousarray(b, dtype=np.float32).reshape(1, 1)
    # wexp[p, s] = W[channel of partition p in slab s] / HW, where the
    # channel of partition p in slab (off, f) is (off + p*f) // HW.
    w_flat = np.asarray(W, dtype=np.float32).reshape(_C)
    p = np.arange(128)[:, None]
    off = np.array([o for o, _ in slabs])[None, :]
    f = np.array([fe for _, fe in slabs])[None, :]
    ch = (off + p * f) // _HW
    wexp = np.ascontiguousarray(w_flat[ch] / np.float32(_HW), dtype=np.float32)
    return [
        {
            "x": np.ascontiguousarray(xs[i * _BPC : (i + 1) * _BPC]),
            "wexp": wexp,
            "bias": b_col,
        }
        for i in range(_NCORES)
    ]


def _gather(results):
    outs = [np.asarray(results[i]["out"]).reshape(_BPC) for i in range(_NCORES)]
    return np.concatenate(outs, axis=0).reshape(_B, 1, 1, 1).astype(np.float32)


def kernel(x, W, b):
    from concourse.bass_utils import run_bass_kernel_spmd

    global _cached_nc
    if _cached_nc is None:
        _cached_nc = _build_nc()
    in_maps = _prepare_in_maps(x, W, b)
    res = run_bass_kernel_spmd(_cached_nc, in_maps, list(range(_NCORES)))
    return _gather(res.results)
